# revision 1
# baseline (speedup 1.0000x reference)
"""Trainium2 Bass kernel for CrossKGAttention (bidirectional masked cross-attention
between two knowledge-graph embedding sets).

Math per direction (queries q_emb [Nq,256], kv kv_emb [Nk,256], mask A [Nq,Nk]):
  Q_i = q_emb @ Wq.T + bq            (head i slice, [Nq,64])
  Kbar = mean_i(kv_emb @ Wk.T + bk)  ([Nk,64])
  V_i  = kv_emb @ Wv.T + bv
  S_i  = Q_i @ Kbar.T * SCALE
  w    = softmax(S_i * A, axis=kv)
  out_i = w @ V_i ;  enhanced = q_emb + out @ Wo.T + bo

Key rewrite used on device: with E = (exp(S)-1) * A  (exactly 0 where A==0),
  unnorm_i = E_i^T-weighted V_i + sum_m V_i[m]     (ones column gives sum_m E)
  denom_i  = Nk + sum_m E_i
  out_i    = unnorm_i / denom_i + bv
All score/exp/PV work happens in the transposed [kv, query] layout so the PV
contraction runs at full PE efficiency with no large transposes; only the tiny
[65, nq] per-head results are transposed back via the PE identity trick.

Sharding: 8 cores; core c owns kg1 query rows [c*750,(c+1)*750) for direction
1->2 and kg2 query rows for 2->1. K/V sources + weights replicated. Queries are
padded 750->768 so every matmul chunk is 256 wide (fp32r full rate, PSUM-bank
aligned).
"""

import numpy as np
import ml_dtypes
from contextlib import ExitStack

import concourse.bass as bass
import concourse.tile as tile
from concourse import bacc, mybir
from concourse.bass_utils import run_bass_kernel_spmd

F32 = mybir.dt.float32
F32R = mybir.dt.float32r
BF16 = mybir.dt.bfloat16
NPBF16 = ml_dtypes.bfloat16

N = 6000          # entities per KG (both sides)
HID = 256
HEADS = 4
D = 64
SCALE = D ** -0.5
NCORES = 8
NQ = N // NCORES          # 750 queries per core per direction
NQP = 768                 # padded queries (3 chunks of 256)
NSZ = 256                 # n-chunk size
NCHUNK = NQP // NSZ       # 3
MBS = 128                 # m-block size
NMB = (N + MBS - 1) // MBS   # 47 (46 full + 112)


def _r32(ap):
    return ap.bitcast(F32R)


def _build_kernel(ctx: ExitStack, tc, ins, outs):
    nc = tc.nc
    (e1T, e2T, eq1T, eq2T, wqT, wkbT, wvT, woT,
     bq_h, bkb, bv2, bo2, a1T, a2T, id128) = ins
    o1T, o2T = outs

    ctx.enter_context(nc.allow_low_precision(reason="fp32r storage is fp32 bits"))
    consts = ctx.enter_context(tc.tile_pool(name="consts", bufs=1))
    perdir = ctx.enter_context(tc.tile_pool(name="perdir", bufs=1))
    small2 = ctx.enter_context(tc.tile_pool(name="small2", bufs=3))
    maskp = ctx.enter_context(tc.tile_pool(name="maskp", bufs=16))
    expp = ctx.enter_context(tc.tile_pool(name="expp", bufs=4))
    ep = ctx.enter_context(tc.tile_pool(name="ep", bufs=4))
    asm = ctx.enter_context(tc.tile_pool(name="asm", bufs=3))
    outp = ctx.enter_context(tc.tile_pool(name="outp", bufs=4))

    # ---- resident constants ----
    wq_sb = consts.tile([128, 2, HID], F32R)
    nc.sync.dma_start(out=wq_sb[:], in_=wqT.rearrange("(b p) h -> p b h", p=128))
    wv_sb = consts.tile([128, 2, HID], F32R)
    nc.sync.dma_start(out=wv_sb[:], in_=wvT.rearrange("(b p) h -> p b h", p=128))
    wo_sb = consts.tile([128, 2, HID], F32R)
    nc.sync.dma_start(out=wo_sb[:], in_=woT.rearrange("(b p) h -> p b h", p=128))
    wkb_sb = consts.tile([128, 2, D], F32R)
    nc.sync.dma_start(out=wkb_sb[:], in_=wkbT.rearrange("(b p) d -> p b d", p=128))
    id_sb = consts.tile([128, 128], F32)
    nc.sync.dma_start(out=id_sb[:], in_=id128[:, :])
    bq_sb = consts.tile([64, HEADS], F32)
    nc.sync.dma_start(out=bq_sb[:], in_=bq_h[:, :])
    bkb_sb = consts.tile([64, 1], F32)
    nc.sync.dma_start(out=bkb_sb[:], in_=bkb[:, :])
    bv_sb = consts.tile([128, 2], F32)
    nc.sync.dma_start(out=bv_sb[:], in_=bv2[:, :])
    bo_sb = consts.tile([128, 2], F32)
    nc.sync.dma_start(out=bo_sb[:], in_=bo2[:, :])

    for dirx in range(2):
        ekvT_d = e2T if dirx == 0 else e1T
        eqT_d = eq1T if dirx == 0 else eq2T
        maskT_d = a1T if dirx == 0 else a2T
        oT_d = o1T if dirx == 0 else o2T

        # ---- load embeddings ----
        ekv_sb = perdir.tile([128, 2, N], F32R, tag="ekv")
        nc.sync.dma_start(out=ekv_sb[:],
                          in_=ekvT_d.rearrange("(b p) m -> p b m", p=128))
        eq_sb = small2.tile([128, 2, NQP], F32R, tag="eq")
        nc.sync.dma_start(out=eq_sb[:],
                          in_=eqT_d.rearrange("(b p) m -> p b m", p=128))

        kb_sb = perdir.tile([64, N], F32R, tag="kb")
        q_sb = perdir.tile([64, HEADS, NQP], F32R, tag="q")
        vt_sb = perdir.tile([128, NMB, HEADS, D + 1], BF16, tag="vt")
        vsum_sb = small2.tile([64, HEADS], F32, tag="vsum")
        es_sb = small2.tile([128, 2], F32R, tag="es")

        with tc.tile_pool(name="projps", bufs=3, space="PSUM") as projps:
            # KbarT projection: [64, N] = wkbT.T @ ekvT  (fp32r)
            for chn in range(N // 500 + 1):
                c0 = chn * 500
                cw = min(500, N - c0)
                if cw <= 0:
                    break
                ps = projps.tile([128, 512], F32, tag="proj")
                for kb in range(2):
                    nc.tensor.matmul(ps[0:64, 0:cw],
                                     _r32(wkb_sb[:, kb, :]),
                                     _r32(ekv_sb[:, kb, c0:c0 + cw]),
                                     start=(kb == 0), stop=(kb == 1))
                nc.vector.tensor_scalar_add(kb_sb[:, c0:c0 + cw],
                                            ps[0:64, 0:cw], bkb_sb[:, 0:1])

            # Q projection per head: [64, NQP]
            for h in range(HEADS):
                for chn in range(2):
                    c0 = chn * 384
                    ps = projps.tile([128, 512], F32, tag="proj")
                    for kb in range(2):
                        nc.tensor.matmul(
                            ps[0:64, 0:384],
                            _r32(wq_sb[:, kb, h * D:(h + 1) * D]),
                            _r32(eq_sb[:, kb, c0:c0 + 384]),
                            start=(kb == 0), stop=(kb == 1))
                    nc.vector.tensor_scalar_add(q_sb[:, h, c0:c0 + 384],
                                                ps[0:64, 0:384],
                                                bq_sb[:, h:h + 1])

            # V projection (natural layout) -> vt_sb bf16 with ones column
            nc.vector.memset(vt_sb[:, :, :, D:D + 1], 1.0)
            for mb in range(NMB):
                m0 = mb * MBS
                mw = min(MBS, N - m0)
                ps = projps.tile([128, 512], F32, tag="proj")
                for kb in range(2):
                    nc.tensor.matmul(ps[0:mw, 0:HID],
                                     _r32(ekv_sb[:, kb, m0:m0 + mw]),
                                     _r32(wv_sb[:, kb, :]),
                                     start=(kb == 0), stop=(kb == 1))
                src = ps[0:mw, 0:HID].rearrange("p (h d) -> p h d", h=HEADS)
                nc.vector.tensor_copy(vt_sb[0:mw, mb, :, 0:D], src)

            # Vsum per head: embsum (DVE free-reduce) then tiny matmuls
            for kb in range(2):
                nc.vector.tensor_reduce(es_sb[:, kb:kb + 1], ekv_sb[:, kb, :],
                                        axis=mybir.AxisListType.X,
                                        op=mybir.AluOpType.add)
            psv = projps.tile([128, 512], F32, tag="proj")
            for h in range(HEADS):
                for kb in range(2):
                    nc.tensor.matmul(psv[0:64, h:h + 1],
                                     wv_sb[:, kb, h * D:(h + 1) * D].bitcast(F32),
                                     es_sb[:, kb:kb + 1].bitcast(F32),
                                     start=(kb == 0), stop=(kb == 1))
            nc.vector.tensor_copy(vsum_sb[:, :], psv[0:64, 0:HEADS])

        oT_sb = perdir.tile([128, 2, NQP], F32R, tag="oT")

        with (tc.tile_pool(name="scrp", bufs=2, space="PSUM") as scrp,
              tc.tile_pool(name="pvp", bufs=1, space="PSUM") as pvp):
          asmps = scrp
          # ---- main loop: per n-chunk of 256 queries ----
          for nt in range(NCHUNK):
            n0 = nt * NSZ
            pv = pvp.tile([D + 1, HEADS, 512], F32, tag="pv")
            for mb in range(NMB):
                m0 = mb * MBS
                mw = min(MBS, N - m0)
                a_t = maskp.tile([128, NSZ], BF16, tag="mask")
                nc.sync.dma_start(out=a_t[0:mw, :],
                                  in_=maskT_d[m0:m0 + mw, n0:n0 + NSZ])
                scr = scrp.tile([128, HEADS, NSZ], F32, tag="scr")
                for h in range(HEADS):
                    nc.tensor.matmul(scr[0:mw, h, :],
                                     _r32(kb_sb[:, m0:m0 + mw]),
                                     _r32(q_sb[:, h, n0:n0 + NSZ]),
                                     start=True, stop=True)
                exp_t = expp.tile([128, HEADS, NSZ], BF16, tag="exp")
                nc.scalar.activation(out=exp_t[0:mw, :, :], in_=scr[0:mw, :, :],
                                     func=mybir.ActivationFunctionType.Exp)
                e_t = ep.tile([128, HEADS, NSZ], BF16, tag="e")
                a_ap = a_t[0:mw, :]
                a_brd = bass.AP(a_ap.tensor, a_ap.offset,
                                [a_ap.ap[0], [0, HEADS], a_ap.ap[1]])
                nc.vector.scalar_tensor_tensor(
                    out=e_t[0:mw, :, :], in0=exp_t[0:mw, :, :], scalar=1.0,
                    in1=a_brd,
                    op0=mybir.AluOpType.subtract, op1=mybir.AluOpType.mult)
                for h in range(HEADS):
                    nc.tensor.matmul(pv[:, h, 0:NSZ],
                                     vt_sb[0:mw, mb, h, :],
                                     e_t[0:mw, h, :],
                                     start=(mb == 0), stop=(mb == NMB - 1))

            # ---- assembly for this n-chunk ----
            p_sb = asm.tile([D + 1, HEADS, NSZ], F32, tag="p")
            for h in range(HEADS):
                nc.vector.tensor_scalar_add(p_sb[0:D, h, :], pv[0:D, h, 0:NSZ],
                                            vsum_sb[:, h:h + 1])
            nc.vector.tensor_scalar_add(p_sb[D:D + 1, :, :],
                                        pv[D:D + 1, :, 0:NSZ], float(N))
            for c in range(2):
                q0 = c * 128
                on_t = asm.tile([128, HEADS, D], F32, tag="onat")
                for h in range(HEADS):
                    trt = asmps.tile([128, HEADS, NSZ], F32, tag="scr")
                    tr = trt[:].rearrange("p a b -> p (a b)")
                    nc.tensor.transpose(tr[0:128, 0:D + 1],
                                        p_sb[0:D + 1, h, q0:q0 + 128],
                                        id_sb[0:D + 1, 0:D + 1])
                    dv = asm.tile([128, 1], F32, tag="dv")
                    nc.vector.reciprocal(dv[:, :], tr[0:128, D:D + 1])
                    nc.vector.tensor_scalar_mul(on_t[:, h, :], tr[0:128, 0:D],
                                                dv[:, 0:1])
                for hb in range(2):
                    trbt = asmps.tile([128, HEADS, NSZ], F32, tag="scr")
                    trb = trbt[:].rearrange("p a b -> p (a b)")
                    srcv = on_t[:].rearrange("p h d -> p (h d)")
                    nc.tensor.transpose(trb[0:128, 0:128],
                                        srcv[:, hb * 128:(hb + 1) * 128],
                                        id_sb[:, :])
                    nc.vector.tensor_scalar_add(
                        oT_sb[:, hb, n0 + q0:n0 + q0 + 128],
                        trb[0:128, 0:128], bv_sb[:, hb:hb + 1])

          # ---- Wo projection + residual ----
          for hb in range(2):
              for chn in range(2):
                  c0 = chn * 384
                  pot = asmps.tile([128, HEADS, NSZ], F32, tag="scr")
                  po = pot[:].rearrange("p a b -> p (a b)")
                  for kb in range(2):
                      nc.tensor.matmul(po[:, 0:384],
                                       _r32(wo_sb[:, kb, hb * 128:(hb + 1) * 128]),
                                       _r32(oT_sb[:, kb, c0:c0 + 384]),
                                       start=(kb == 0), stop=(kb == 1))
                  enh = outp.tile([128, 384], F32, tag="enh")
                  nc.vector.scalar_tensor_tensor(
                      out=enh[:, :], in0=po[:, 0:384], scalar=bo_sb[:, hb:hb + 1],
                      in1=eq_sb[:, hb, c0:c0 + 384],
                      op0=mybir.AluOpType.add, op1=mybir.AluOpType.add)
                  nc.sync.dma_start(
                      out=oT_d.rearrange("(b p) m -> p b m", p=128)[:, hb, c0:c0 + 384],
                      in_=enh[:, :])


def _build_program():
    nc = bacc.Bacc("TRN2", target_bir_lowering=False, debug=False,
                   num_devices=NCORES)

    def din(name, shape, dt):
        return nc.dram_tensor(name, shape, dt, kind="ExternalInput").ap()

    ins = [
        din("e1T", [HID, N], F32R),
        din("e2T", [HID, N], F32R),
        din("eq1T", [HID, NQP], F32R),
        din("eq2T", [HID, NQP], F32R),
        din("wqT", [HID, HID], F32R),
        din("wkbT", [HID, D], F32R),
        din("wvT", [HID, HID], F32R),
        din("woT", [HID, HID], F32R),
        din("bq_h", [64, HEADS], F32),
        din("bkb", [64, 1], F32),
        din("bv2", [128, 2], F32),
        din("bo2", [128, 2], F32),
        din("a1T", [N, NQP], BF16),
        din("a2T", [N, NQP], BF16),
        din("id128", [128, 128], F32),
    ]
    outs = [
        nc.dram_tensor("o1T", [HID, NQP], F32, kind="ExternalOutput").ap(),
        nc.dram_tensor("o2T", [HID, NQP], F32, kind="ExternalOutput").ap(),
    ]
    with tile.TileContext(nc) as tc:
        with ExitStack() as ctx:
            _build_kernel(ctx, tc, ins, outs)
    nc.compile()
    return nc


_NC_CACHE = None
LAST_RESULTS = None


def kernel(kg1_emb, kg2_emb, alignment_matrix, Wq, bq, Wk, bk, Wv, bv, Wo, bo):
    global _NC_CACHE
    kg1 = np.asarray(kg1_emb, np.float32)
    kg2 = np.asarray(kg2_emb, np.float32)
    align = np.asarray(alignment_matrix, np.float32)
    Wq = np.asarray(Wq, np.float32); bq = np.asarray(bq, np.float32)
    Wk = np.asarray(Wk, np.float32); bk = np.asarray(bk, np.float32)
    Wv = np.asarray(Wv, np.float32); bv = np.asarray(bv, np.float32)
    Wo = np.asarray(Wo, np.float32); bo = np.asarray(bo, np.float32)

    # host-side layout prep (no reference math beyond weight folding of the
    # head-mean + scale, which is a constant-folding rewrite of the same graph)
    e1T = np.ascontiguousarray(kg1.T)
    e2T = np.ascontiguousarray(kg2.T)
    Wkb = Wk.reshape(HEADS, D, HID).mean(axis=0) * SCALE     # [64, 256]
    bkbv = (bk.reshape(HEADS, D).mean(axis=0) * SCALE).reshape(64, 1)
    wqT = np.ascontiguousarray(Wq.T)
    wkbT = np.ascontiguousarray(Wkb.T)
    wvT = np.ascontiguousarray(Wv.T)
    woT = np.ascontiguousarray(Wo.T)
    bq_h = np.ascontiguousarray(bq.reshape(HEADS, D).T)
    bv2 = np.ascontiguousarray(bv.reshape(2, 128).T)
    bo2 = np.ascontiguousarray(bo.reshape(2, 128).T)
    id128 = np.eye(128, dtype=np.float32)

    alignT_b = np.ascontiguousarray(align.T).astype(NPBF16)   # [m2, n1]
    align_b = align.astype(NPBF16)                            # [m1, n2]

    if _NC_CACHE is None:
        _NC_CACHE = _build_program()
    nc = _NC_CACHE

    in_maps = []
    for c in range(NCORES):
        r0 = c * NQ
        eq1 = np.zeros((HID, NQP), np.float32)
        eq1[:, 0:NQ] = e1T[:, r0:r0 + NQ]
        eq2 = np.zeros((HID, NQP), np.float32)
        eq2[:, 0:NQ] = e2T[:, r0:r0 + NQ]
        a1 = np.zeros((N, NQP), NPBF16)
        a1[:, 0:NQ] = alignT_b[:, r0:r0 + NQ]
        a2 = np.zeros((N, NQP), NPBF16)
        a2[:, 0:NQ] = align_b[:, r0:r0 + NQ]
        in_maps.append({
            "e1T": e1T, "e2T": e2T, "eq1T": eq1, "eq2T": eq2,
            "wqT": wqT, "wkbT": wkbT, "wvT": wvT, "woT": woT,
            "bq_h": bq_h, "bkb": bkbv, "bv2": bv2, "bo2": bo2,
            "a1T": a1, "a2T": a2, "id128": id128,
        })

    import os
    trace = os.environ.get("CKG_TRACE", "0") == "1"
    res = run_bass_kernel_spmd(nc, in_maps, core_ids=list(range(NCORES)),
                               trace=trace)
    global LAST_RESULTS
    LAST_RESULTS = res

    kg1_out = np.empty((N, HID), np.float32)
    kg2_out = np.empty((N, HID), np.float32)
    for c in range(NCORES):
        r0 = c * NQ
        kg1_out[r0:r0 + NQ, :] = res.results[c]["o1T"][:, 0:NQ].T
        kg2_out[r0:r0 + NQ, :] = res.results[c]["o2T"][:, 0:NQ].T
    return (kg1_out, kg2_out)

